# revision 24
# baseline (speedup 1.0000x reference)
"""Trainium2 Bass kernel for CenterDependentPool2D.

Input  x: (8, 64, 448, 448) fp32  ->  Output: (8, 64, 224, 224) fp32.

Strategy (per core = one batch element, 64 channels):
  - Partition p = c + 64*wg: channel c, wg = column half (0: out cols 0..111,
    1: out cols 112..223). Both pooling axes live in the free dimension.
  - All five ring windows (k in {2,8,14,20,26}, stride 2, reflect pad)
    decompose over pair-max arrays (even pairs E / odd pairs O in both axes).
    Ring r's window is an s x s stride-1 square over EE (s=1/7/13) or OO
    (s=4/10), via a shared shifted-max pyramid (S2->S4->S8 + final combine).
  - Reflect padding == window clipping here, so out-of-range leaves are
    -BIG fills.
  - 32-row output bands (8 bands): amortizes the +/-6-row pyramid halos and
    per-instruction overhead (~170ns/op on DVE) twice as well as 16-row
    bands.  Input lands in 16-row chunks (4 DMA chunks/band, 2 rotating
    landing buffers) to keep SBUF under budget.
  - EXTENT SPECIALIZATION: per band, each ring only exists on small column
    intervals per partition group (annulus geometry).  Every pyramid stage
    is emitted only on the column/row intervals its consumers need
    (propagated backward per stage in _plan_bands so producers and
    consumers are exactly consistent).  Per-group ops are merged into one
    128-partition op whenever the union costs less than an extra
    instruction.  The blend picks the area-max region as in-place base
    (ee centrally, s13 at edges) and overwrites the other (disjoint)
    annulus regions with extent-limited copy_predicated ops.
  - Pipeline in fp16 after the first max (monotone rounding); final cast
    back to fp32 on the scalar engine.
"""

import numpy as np

import concourse.bass as bass
import concourse.mybir as mybir
from concourse.tile import TileContext
from concourse.bass_utils import run_bass_kernel_spmd

# ---------------- problem constants ----------------
B, C, IN, OUT = 8, 64, 448, 224
OW = 112          # out cols per wg
EW = 124          # e-column count of pair arrays
WIN = 250         # input tile cols (incl pads)
NEG = -30000.0    # "minus infinity" that survives fp16
R2 = [60 * 60, 75 * 75, 90 * 90, 105 * 105]
BH = 32           # output rows per band
NB = 8            # number of row bands

DT = mybir.dt.float16      # pipeline dtype after first max

_CACHED = {}


def _regions():
    """Disjoint region maps [5][224][224] u8: 0=disk60, 1=ann60-75,
    2=ann75-90, 3=ann90-105, 4=out105.  Sources: 0->ee(k=2), 1->s4o(k=8),
    2->s7(k=14), 3->s10(k=20), 4->s13(k=26)."""
    yy, xx = np.mgrid[0:OUT, 0:OUT]
    d2 = (yy - OUT // 2) ** 2 + (xx - OUT // 2) ** 2
    d60, d75, d90, d105 = (d2 < r for r in R2)
    return np.stack([d60, d75 & ~d60, d90 & ~d75, d105 & ~d90,
                     ~d105]).astype(np.uint8)


def _ival(mask2d):
    if not mask2d.any():
        return None
    rows = np.nonzero(mask2d.any(axis=1))[0]
    cols = np.nonzero(mask2d.any(axis=0))[0]
    return (int(rows[0]), int(rows[-1]) + 1, int(cols[0]), int(cols[-1]) + 1)


def _union(a, b):
    if a is None:
        return b
    if b is None:
        return a
    return (min(a[0], b[0]), max(a[1], b[1]),
            min(a[2], b[2]), max(a[3], b[3]))


def _plan_bands():
    regs = _regions()
    plans = []
    for it in range(NB):
        y0 = max(0, BH * it - 8)
        y1 = min(OUT, BH * it + BH - 8)
        H = y1 - y0
        ext = [[_ival(regs[r, y0:y1, wg * OW:(wg + 1) * OW])
                for wg in range(2)] for r in range(5)]
        present = [r for r in range(5) if any(e is not None for e in ext[r])]
        base = 0 if 0 in present else 4
        cps = [r for r in present if r != base]

        p = dict(it=it, y0=y0, y1=y1, H=H, ext=ext, base=base, cps=cps)

        # ---------- EE-side intervals (per wg) ----------
        if base == 4:
            s13 = [(0, H, 0, OW), (0, H, 0, OW)]
        else:
            s13 = [ext[4][wg] for wg in range(2)]
        v = [None if iv is None else
             (iv[0], iv[1], iv[2], min(iv[3] + 5, 117)) for iv in s13]
        s8 = [None if iv is None else
              (iv[0], min(iv[1] + 5, H + 5), iv[2], iv[3]) for iv in v]
        a8 = [None if iv is None else
              (iv[0], iv[1], iv[2], min(iv[3] + 4, 121)) for iv in s8]
        s7 = [None if ext[2][wg] is None else
              (ext[2][wg][0], ext[2][wg][1],
               ext[2][wg][2] + 3, ext[2][wg][3] + 3) for wg in range(2)]
        u = [None if iv is None else
             (iv[0], iv[1], iv[2], min(iv[3] + 3, 121)) for iv in s7]
        s4t = [None, None]
        for wg in range(2):
            for iv in (a8[wg], u[wg]):
                if iv is not None:
                    s4t[wg] = _union(s4t[wg],
                                     (0, H + 9, iv[2], min(iv[3], 121)))
        a4 = [None if iv is None else
              (0, H + 9, iv[2], min(iv[3] + 2, 123)) for iv in s4t]
        a2 = [None if iv is None else
              (0, H + 11, iv[2], min(iv[3] + 1, EW)) for iv in a4]

        # ---------- OO-side intervals (per wg) ----------
        s10 = [None if ext[3][wg] is None else
               (ext[3][wg][0], ext[3][wg][1],
                ext[3][wg][2] + 1, ext[3][wg][3] + 1) for wg in range(2)]
        w = [None if iv is None else
             (iv[0], iv[1], iv[2], min(iv[3] + 2, 117)) for iv in s10]
        s8o = [None if iv is None else
               (iv[0], min(iv[1] + 2, H + 2), iv[2], iv[3]) for iv in w]
        a8o = [None if iv is None else
               (iv[0], iv[1], iv[2], min(iv[3] + 4, 121)) for iv in s8o]
        s4oS = [None if ext[1][wg] is None else
                (ext[1][wg][0], ext[1][wg][1],
                 ext[1][wg][2] + 4, ext[1][wg][3] + 4) for wg in range(2)]
        s4o = [None, None]
        for wg in range(2):
            for iv in (s4oS[wg], a8o[wg]):
                if iv is not None:
                    s4o[wg] = _union(s4o[wg],
                                     (0, H + 6, iv[2], min(iv[3], 121)))
        a4o = [None if iv is None else
               (0, H + 6, iv[2], min(iv[3] + 2, 123)) for iv in s4o]
        a2o = [None if iv is None else
               (0, H + 8, iv[2], min(iv[3] + 1, EW)) for iv in a4o]
        oo_c = None
        for iv in a2o:
            if iv is not None:
                oo_c = _union(oo_c, iv)
        oo_c = None if oo_c is None else (oo_c[2], oo_c[3])

        p.update(s13=s13, v=v, s8=s8, a8=a8, s7=s7, u=u, s4t=s4t, a4=a4,
                 a2=a2, s10=s10, w=w, s8o=s8o, a8o=a8o, s4o=s4o, a4o=a4o,
                 a2o=a2o, oo_c=oo_c)
        plans.append(p)
    return plans


_PLANS = _plan_bands()


def _build_masks():
    """bm [128, NB, 4, BH, OW] u8: per band, cp-plane region masks."""
    regs = _regions()
    bm = np.zeros((128, NB, 4, BH, OW), np.uint8)
    for pl in _PLANS:
        it, y0, y1 = pl["it"], pl["y0"], pl["y1"]
        for i, r in enumerate(pl["cps"]):
            for wg in range(2):
                sl = regs[r, y0:y1, wg * OW:(wg + 1) * OW]
                bm[wg * 64:(wg + 1) * 64, it, i, 0:y1 - y0, :] = sl[None]
    return bm


def split_multi_waits(nc):
    """walrus CoreV3Gen accepts at most 1 sync-wait per instruction; Tile's
    tail drains can carry 2+.  Peel extras onto preceding NoOps."""
    n = 0
    for fn in nc.m.functions:
        for bb in fn.blocks:
            insts = list(bb.instructions)
            out = []
            for ins in insts:
                si = getattr(ins, "sync_info", None)
                if si is not None and len(si.on_wait) > 1:
                    waits = list(si.on_wait)
                    for k, w in enumerate(waits[:-1]):
                        nop = mybir.InstNoOp(
                            name=f"{ins.name}-waitsplit{k}",
                            engine=ins.engine, ins=[], outs=[])
                        nop.sync_info = mybir.SyncInfo(on_wait=[w], on_update=[])
                        out.append(nop)
                        n += 1
                    ins.sync_info = mybir.SyncInfo(
                        on_wait=[waits[-1]], on_update=list(si.on_update))
                out.append(ins)
            if n:
                bb.instructions = out
    return n


def _emit_kernel(nc: bass.Bass):
    x = nc.dram_tensor("x", [C, IN, IN], mybir.dt.float32, kind="ExternalInput")
    y = nc.dram_tensor("y", [C, OUT, OUT], mybir.dt.float32, kind="ExternalOutput")
    rmask = nc.inline_tensor(_build_masks(), name="rmask")

    dve = nc.vector
    act = nc.scalar
    mx = mybir.AluOpType.max

    with TileContext(nc) as tc:
        with tc.tile_pool(name="pp", bufs=1) as pers, \
             tc.tile_pool(name="tp", bufs=1) as tP, \
             tc.tile_pool(name="tq", bufs=1) as tQ, \
             tc.tile_pool(name="tr", bufs=2) as tR, \
             tc.tile_pool(name="to", bufs=1) as tPo, \
             tc.tile_pool(name="tqo", bufs=1) as tQo:

            # 16-row landing chunks (fp32) + fp16 cast staging
            it_bufs = [pers.tile([128, 16, WIN], mybir.dt.float32, tag=f"in{i}", name=f"itile{i}")
                       for i in range(2)]
            # deinterleaved fp16 cast staging: even cols T[2k], odd T[2k+1]
            xfe_bufs = [pers.tile([128, 16, 125], DT, tag=f"xfe{i}", name=f"xfe{i}")
                        for i in range(2)]
            xfo_bufs = [pers.tile([128, 16, 125], DT, tag=f"xfo{i}", name=f"xfo{i}")
                        for i in range(2)]
            ewt = pers.tile([128, 92, EW], DT, tag="ewt")
            owt = pers.tile([128, 92, EW], DT, tag="owt")
            ee = pers.tile([128, 45, EW], DT, tag="ee")
            oo = pers.tile([128, 43, EW], DT, tag="oo")
            s4t_t = pers.tile([128, BH + 9, 121], DT, tag="s4")
            s4o_t = pers.tile([128, BH + 6, 121], DT, tag="s4o")
            mask_t = pers.tile([128, 4, BH, OW], mybir.dt.uint8, tag="mk")
            out_bufs = [pers.tile([128, 8, OW], mybir.dt.float32, tag=f"out{i}",
                                  name=f"outt{i}") for i in range(2)]

            for itile in it_bufs:
                nc.gpsimd.memset(itile[0:64, :, 0:13], NEG)
                nc.gpsimd.memset(itile[64:128, :, 237:WIN], NEG)

            def front_thunks(it):
                """Roll + input DMAs + stage-1 pair-max + ee/oo for band
                `it`, as an ordered thunk list (DMAs interleaved ahead of
                their consumers so prefetch depth stays >= 2 chunks)."""
                pl = _PLANS[it]
                ow_iv = pl["oo_c"]
                if it + 1 < NB and _PLANS[it + 1]["oo_c"] is not None:
                    n0, n1 = _PLANS[it + 1]["oo_c"]
                    ow_iv = (min(ow_iv[0], n0), max(ow_iv[1], n1)) \
                        if ow_iv is not None else (n0, n1)

                th = []

                def roll():
                    if it == 0:
                        nc.gpsimd.memset(ewt[:, 0:28, :], NEG)
                        nc.gpsimd.memset(owt[:, 0:28, :], NEG)
                    else:
                        act.copy(ewt[:, 0:28, :], ewt[:, 64:92, :])
                        if pl["oo_c"] is not None:
                            rc0, rc1 = pl["oo_c"]
                            act.copy(owt[:, 0:28, rc0:rc1],
                                     owt[:, 64:92, rc0:rc1])
                if it == 0:
                    th.append(roll)

                def dma(ch):
                    r0 = 64 * it + 16 * ch
                    itile = it_bufs[ch % 2]
                    nc.sync.dma_start(itile[0:64, :, 13:WIN],
                                      x[:, r0:r0 + 16, 0:237])
                    nc.sync.dma_start(itile[64:128, :, 1:237],
                                      x[:, r0:r0 + 16, 212:448])

                def cast(ch):
                    # deinterleaving cast on ACT: strided reads cost the same
                    # there, and make every DVE consumer packed (2x mode)
                    act.copy(xfe_bufs[ch % 2][:, :, :],
                             it_bufs[ch % 2][:, :, 0:250:2])
                    act.copy(xfo_bufs[ch % 2][:, :, :],
                             it_bufs[ch % 2][:, :, 1:250:2])

                def comp(ch):
                    xfe = xfe_bufs[ch % 2]
                    xfo = xfo_bufs[ch % 2]
                    e0 = 28 + 16 * ch
                    # Ew[e] = max(T[2e+1], T[2e+2]) = max(odd[e], even[e+1])
                    dve.tensor_tensor(ewt[:, e0:e0 + 16, :],
                                      xfo[:, :, 0:124],
                                      xfe[:, :, 1:125], mx)
                    if ow_iv is not None:
                        c0, c1 = ow_iv
                        # Ow[e] = max(T[2e+2], T[2e+3]) = max(even[e+1], odd[e+1])
                        dve.tensor_tensor(owt[:, e0:e0 + 16, c0:c1],
                                          xfe[:, :, 1 + c0:1 + c1],
                                          xfo[:, :, 1 + c0:1 + c1], mx)

                def comp32(ch):
                    # band 0 ramp: DVE is DMA-starved anyway, so read the
                    # fp32 landing tile directly (1x) and skip the cast hop
                    itile = it_bufs[ch % 2]
                    e0 = 28 + 16 * ch
                    dve.tensor_tensor(ewt[:, e0:e0 + 16, :],
                                      itile[:, :, 1:249:2],
                                      itile[:, :, 2:250:2], mx)
                    if ow_iv is not None:
                        c0, c1 = ow_iv
                        dve.tensor_tensor(
                            owt[:, e0:e0 + 16, c0:c1],
                            itile[:, :, 2 + 2 * c0:2 + 2 * c1:2],
                            itile[:, :, 3 + 2 * c0:3 + 2 * c1:2], mx)

                rest = []
                if it == 0:
                    th.append(lambda: dma(0))
                    th.append(lambda: dma(1))
                    rest.append(lambda: comp32(0))
                    rest.append(lambda: dma(2))
                    rest.append(lambda: comp32(1))
                    rest.append(lambda: dma(3))
                    rest.append(lambda: comp32(2))
                    rest.append(lambda: comp32(3))
                elif it < 7:
                    th.append(lambda: dma(0))
                    th.append(lambda: dma(1))
                    rest.append(lambda: cast(0))
                    rest.append(lambda: dma(2))
                    rest.append(lambda: comp(0))
                    rest.append(lambda: cast(1))
                    rest.append(lambda: dma(3))
                    rest.append(lambda: comp(1))
                    rest.append(lambda: cast(2))
                    rest.append(roll)
                    rest.append(lambda: comp(2))
                    rest.append(lambda: cast(3))
                    rest.append(lambda: comp(3))
                else:
                    th.append(roll)
                    def negfill():
                        # band 7 only reads ewt rows <40 / owt rows <36
                        nc.gpsimd.memset(ewt[:, 28:40, :], NEG)
                        nc.gpsimd.memset(owt[:, 28:36, :], NEG)
                    th.append(negfill)

                def eeoo():
                    if it == 0:
                        nc.gpsimd.memset(ee[:, 0:14, :], NEG)
                        nc.gpsimd.memset(oo[:, 0:14, :], NEG)
                        rlo = 14
                    else:
                        rlo = 0
                    # a2 reads ee rows < y1+6-J0; a2o reads oo rows < y1+4-J0
                    J0 = BH * it - 14
                    eh = min(45, pl["y1"] + 6 - J0)
                    oh = min(43, pl["y1"] + 4 - J0)
                    dve.tensor_tensor(ee[:, rlo:eh, :],
                                      ewt[:, 2 * rlo:2 * eh:2, :],
                                      ewt[:, 2 * rlo + 1:2 * eh + 1:2, :], mx)
                    if pl["oo_c"] is not None:
                        c0, c1 = pl["oo_c"]
                        dve.tensor_tensor(
                            oo[:, rlo:oh, c0:c1],
                            owt[:, 2 * rlo + 1:2 * oh + 1:2, c0:c1],
                            owt[:, 2 * rlo + 2:2 * oh + 2:2, c0:c1], mx)
                return th, rest, eeoo

            def rcomb(dst, dst_base, src, src_base, jlo, jhi, d,
                      clo, chi, p0, p1):
                a = jlo - src_base
                b = jhi - src_base
                o0 = jlo - dst_base
                o1 = jhi - dst_base
                dve.tensor_tensor(dst[p0:p1, o0:o1, clo:chi],
                                  src[p0:p1, a:b, clo:chi],
                                  src[p0:p1, a + d:b + d, clo:chi], mx)

            def ccomb(dst, src, r0_, r1_, d, clo, chi, p0, p1):
                dve.tensor_tensor(dst[p0:p1, r0_:r1_, clo:chi],
                                  src[p0:p1, r0_:r1_, clo:chi],
                                  src[p0:p1, r0_:r1_, clo + d:chi + d], mx)

            def per_wg(th, ivs, fn, ovh_elems=330):
                """Append per-group emission thunks (merged into one
                128-partition op when the union is cheaper)."""
                def emit():
                    if ivs[0] is None or ivs[1] is None:
                        for wg in range(2):
                            if ivs[wg] is not None:
                                fn(wg * 64, wg * 64 + 64, ivs[wg])
                        return
                    if ivs[0] == ivs[1]:
                        fn(0, 128, ivs[0])
                        return
                    un = _union(ivs[0], ivs[1])
                    c_merge = (un[1] - un[0]) * (un[3] - un[2])
                    c_split = sum((iv[1] - iv[0]) * (iv[3] - iv[2])
                                  for iv in ivs) + ovh_elems
                    if c_merge < c_split:
                        fn(0, 128, un)
                    else:
                        for wg in range(2):
                            fn(wg * 64, wg * 64 + 64, ivs[wg])
                th.append(emit)

            def pyramid_thunks(it):
                pl = _PLANS[it]
                y0, y1, H = pl["y0"], pl["y1"], pl["H"]
                J0 = BH * it - 14
                base, cps = pl["base"], pl["cps"]
                th = []

                # ================= EE-side pyramid =================
                a2 = tP.tile([128, BH + 11, EW], DT, tag="p0")
                s2 = tQ.tile([128, BH + 11, 123], DT, tag="q0")
                a4 = tP.tile([128, BH + 11, 123], DT, tag="p0")

                per_wg(th, pl["a2"], lambda p0, p1, iv:
                       rcomb(a2, y0 - 6, ee, J0, y0 - 6, y1 + 5, 1,
                             iv[2], iv[3], p0, p1))
                per_wg(th, pl["a4"], lambda p0, p1, iv:
                       ccomb(s2, a2, 0, H + 11, 1, iv[2], iv[3], p0, p1))
                per_wg(th, pl["a4"], lambda p0, p1, iv:
                       rcomb(a4, y0 - 6, s2, y0 - 6, y0 - 6, y1 + 3, 2,
                             iv[2], iv[3], p0, p1))
                per_wg(th, pl["s4t"], lambda p0, p1, iv:
                       ccomb(s4t_t, a4, 0, H + 9, 2, iv[2], iv[3], p0, p1))

                u_t = tR.tile([128, BH, 121], DT, tag="r0")
                s7_t = tR.tile([128, BH, 118], DT, tag="r0")
                per_wg(th, pl["u"], lambda p0, p1, iv:
                       rcomb(u_t, y0 - 3, s4t_t, y0 - 6, y0 - 3 + iv[0],
                             y0 - 3 + iv[1], 3, iv[2], iv[3], p0, p1))
                per_wg(th, pl["s7"], lambda p0, p1, iv:
                       ccomb(s7_t, u_t, iv[0], iv[1], 3, iv[2],
                             min(iv[3], 118), p0, p1))

                a8_t = tP.tile([128, BH + 11, 121], DT, tag="p0")
                s8_t = tQ.tile([128, BH + 11, 117], DT, tag="q0")
                v_t = tP.tile([128, BH + 11, 117], DT, tag="p0")
                s13_t = tR.tile([128, BH, 112], DT, tag="r0")

                per_wg(th, pl["a8"], lambda p0, p1, iv:
                       rcomb(a8_t, y0 - 6, s4t_t, y0 - 6, y0 - 6 + iv[0],
                             y0 - 6 + iv[1], 4, iv[2], iv[3], p0, p1))
                per_wg(th, pl["s8"], lambda p0, p1, iv:
                       ccomb(s8_t, a8_t, iv[0], iv[1], 4, iv[2], iv[3],
                             p0, p1))
                per_wg(th, pl["v"], lambda p0, p1, iv:
                       rcomb(v_t, y0 - 6, s8_t, y0 - 6, y0 - 6 + iv[0],
                             y0 - 6 + iv[1], 5, iv[2], iv[3], p0, p1))
                per_wg(th, pl["s13"], lambda p0, p1, iv:
                       ccomb(s13_t, v_t, iv[0], iv[1], 5, iv[2],
                             min(iv[3], 112), p0, p1))

                # ================= OO-side pyramid =================
                s10_t = s7x = None
                if any(iv is not None for iv in pl["s4o"]):
                    a2o = tPo.tile([128, BH + 8, EW], DT, tag="po")
                    s2o = tQo.tile([128, BH + 8, 123], DT, tag="qo")
                    a4o = tPo.tile([128, BH + 8, 123], DT, tag="po")

                    per_wg(th, pl["a2o"], lambda p0, p1, iv:
                           rcomb(a2o, y0 - 5, oo, J0, y0 - 5, y1 + 3, 1,
                                 iv[2], iv[3], p0, p1))
                    per_wg(th, pl["a4o"], lambda p0, p1, iv:
                           ccomb(s2o, a2o, 0, H + 8, 1, iv[2], iv[3], p0, p1))
                    per_wg(th, pl["a4o"], lambda p0, p1, iv:
                           rcomb(a4o, y0 - 5, s2o, y0 - 5, y0 - 5, y1 + 1, 2,
                                 iv[2], iv[3], p0, p1))
                    per_wg(th, pl["s4o"], lambda p0, p1, iv:
                           ccomb(s4o_t, a4o, 0, H + 6, 2, iv[2], iv[3],
                                 p0, p1))

                if any(iv is not None for iv in pl["s10"]):
                    a8o = tPo.tile([128, BH + 8, 121], DT, tag="po")
                    s8o = tQo.tile([128, BH + 8, 117], DT, tag="qo")
                    w_t = tPo.tile([128, BH + 8, 117], DT, tag="po")
                    s10_t = tQo.tile([128, BH + 8, 115], DT, tag="qo")

                    per_wg(th, pl["a8o"], lambda p0, p1, iv:
                           rcomb(a8o, y0 - 5, s4o_t, y0 - 5, y0 - 5 + iv[0],
                                 y0 - 5 + iv[1], 4, iv[2], iv[3], p0, p1))
                    per_wg(th, pl["s8o"], lambda p0, p1, iv:
                           ccomb(s8o, a8o, iv[0], iv[1], 4, iv[2], iv[3],
                                 p0, p1))
                    per_wg(th, pl["w"], lambda p0, p1, iv:
                           rcomb(w_t, y0 - 5, s8o, y0 - 5, y0 - 5 + iv[0],
                                 y0 - 5 + iv[1], 2, iv[2], iv[3], p0, p1))
                    per_wg(th, pl["s10"], lambda p0, p1, iv:
                           ccomb(s10_t, w_t, iv[0], iv[1], 2, iv[2],
                                 min(iv[3], 115), p0, p1))

                # ---- masks DMA ----
                ncp = len(cps)
                if ncp:
                    th.append(lambda: nc.sync.dma_start(
                        mask_t[:, 0:ncp, 0:H, :],
                        rmask[:, it, 0:ncp, 0:H, :]))

                # ---- blend ----
                if base == 0:
                    acc_tile, acc_r0, acc_c0 = ee, y0 - J0, 6
                else:
                    acc_tile, acc_r0, acc_c0 = s13_t, 0, 0

                srcs = {0: (ee, y0 - J0, 6), 1: (s4o_t, 3, 4),
                        2: (s7_t, 0, 3), 3: (s10_t, 0, 1), 4: (s13_t, 0, 0)}
                for i, r in enumerate(cps):
                    stile, sr0, sc0 = srcs[r]

                    def em_cp(p0, p1, iv, i=i, stile=stile, sr0=sr0, sc0=sc0):
                        r0_, r1_, c0_, c1_ = iv
                        dve.copy_predicated(
                            acc_tile[p0:p1, acc_r0 + r0_:acc_r0 + r1_,
                                     acc_c0 + c0_:acc_c0 + c1_],
                            mask_t[p0:p1, i, r0_:r1_, c0_:c1_],
                            stile[p0:p1, sr0 + r0_:sr0 + r1_,
                                  sc0 + c0_:sc0 + c1_])

                    per_wg(th, pl["ext"][r], em_cp, ovh_elems=165)

                # ---- cast + store (16-row halves) ----
                def cast_store():
                    for k, h0 in enumerate(range(0, H, 8)):
                        h1 = min(h0 + 8, H)
                        out_t = out_bufs[k % 2]
                        acc = acc_tile[:, acc_r0 + h0:acc_r0 + h1,
                                       acc_c0:acc_c0 + OW]
                        act.copy(out_t[:, 0:h1 - h0, :], acc)
                        yv = y[:, y0 + h0:y0 + h1, :].rearrange(
                            "c h (w o) -> w c h o", o=OW)
                        nc.sync.dma_start(yv[0], out_t[0:64, 0:h1 - h0, :])
                        nc.sync.dma_start(yv[1], out_t[64:128, 0:h1 - h0, :])
                th.append(cast_store)
                return th

            # ---- software-pipelined emission: band it's input pipeline is
            # interleaved into band it-1's pyramid so chunk DMAs always have
            # pyramid work to hide behind ----
            lead0, rest0, eeoo0 = front_thunks(0)
            for t in lead0 + rest0:
                t()
            eeoo0()
            for it in range(1, NB):
                # ee/oo(it) overwrite tiles still read/written by band it-1's
                # pyramid+blend+cast: they must come strictly after it.
                lead, rest, eeoo = front_thunks(it)
                for t in lead:
                    t()
                pyr = pyramid_thunks(it - 1)
                # front-load the input-pipeline thunks: casts/DMAs early so
                # ACT finishes casts well before the DVE reaches the comps;
                # comps spread over the remaining pyramid.
                nfront = min(len(rest), max(0, len(pyr) - 1))
                half = (len(rest) + 1) // 2
                pos = []
                for j in range(len(rest)):
                    if j < half:
                        pos.append(2 * (j + 1))          # every 2nd slot
                    else:
                        span = max(1, len(pyr) - 2 * half - 2)
                        pos.append(2 * half +
                                   (j - half + 1) * span // (len(rest) - half + 1))
                fi = 0
                for i, t in enumerate(pyr):
                    t()
                    while fi < len(rest) and fi < len(pos) and pos[fi] <= i:
                        rest[fi]()
                        fi += 1
                while fi < len(rest):
                    rest[fi]()
                    fi += 1
                eeoo()
            for t in pyramid_thunks(NB - 1):
                t()

    return nc


def _get_nc():
    if "nc" not in _CACHED:
        nc = bass.Bass()
        _emit_kernel(nc)
        split_multi_waits(nc)
        _CACHED["nc"] = nc
    return _CACHED["nc"]


def kernel(x: np.ndarray) -> np.ndarray:
    nc = _get_nc()
    in_maps = [{"x": np.ascontiguousarray(x[b], dtype=np.float32)}
               for b in range(B)]
    res = run_bass_kernel_spmd(nc, in_maps, core_ids=list(range(B)))
    return np.stack([r["y"] for r in res.results]).astype(np.float32)


# revision 25
# speedup vs baseline: 1.1917x; 1.1917x over previous
"""Trainium2 Bass kernel for CenterDependentPool2D.

Input  x: (8, 64, 448, 448) fp32  ->  Output: (8, 64, 224, 224) fp32.

Strategy (per core = one batch element, 64 channels):
  - Partition p = c + 64*wg: channel c, wg = column half (0: out cols 0..111,
    1: out cols 112..223). Both pooling axes live in the free dimension.
  - All five ring windows (k in {2,8,14,20,26}, stride 2, reflect pad)
    decompose over pair-max arrays (even pairs E / odd pairs O in both axes).
    Ring r's window is an s x s stride-1 square over EE (s=1/7/13) or OO
    (s=4/10), via a shared shifted-max pyramid (S2->S4->S8 + final combine).
  - Reflect padding == window clipping here, so out-of-range leaves are
    -BIG fills.
  - 32-row output bands (8 bands): amortizes the +/-6-row pyramid halos and
    per-instruction overhead (~170ns/op on DVE) twice as well as 16-row
    bands.  Input lands in 16-row chunks (4 DMA chunks/band, 2 rotating
    landing buffers) to keep SBUF under budget.
  - EXTENT SPECIALIZATION: per band, each ring only exists on small column
    intervals per partition group (annulus geometry).  Every pyramid stage
    is emitted only on the column/row intervals its consumers need
    (propagated backward per stage in _plan_bands so producers and
    consumers are exactly consistent).  Per-group ops are merged into one
    128-partition op whenever the union costs less than an extra
    instruction.  The blend picks the area-max region as in-place base
    (ee centrally, s13 at edges) and overwrites the other (disjoint)
    annulus regions with extent-limited copy_predicated ops.
  - Pipeline in fp16 after the first max (monotone rounding); final cast
    back to fp32 on the scalar engine.
"""

import numpy as np

import concourse.bass as bass
import concourse.mybir as mybir
from concourse.tile import TileContext
from concourse.bass_utils import run_bass_kernel_spmd

# ---------------- problem constants ----------------
B, C, IN, OUT = 8, 64, 448, 224
OW = 112          # out cols per wg
EW = 124          # e-column count of pair arrays
WIN = 250         # input tile cols (incl pads)
NEG = -30000.0    # "minus infinity" that survives fp16
R2 = [60 * 60, 75 * 75, 90 * 90, 105 * 105]
BH = 32           # output rows per band
NB = 8            # number of row bands

DT = mybir.dt.float16      # pipeline dtype after first max

_CACHED = {}


def _regions():
    """Disjoint region maps [5][224][224] u8: 0=disk60, 1=ann60-75,
    2=ann75-90, 3=ann90-105, 4=out105.  Sources: 0->ee(k=2), 1->s4o(k=8),
    2->s7(k=14), 3->s10(k=20), 4->s13(k=26)."""
    yy, xx = np.mgrid[0:OUT, 0:OUT]
    d2 = (yy - OUT // 2) ** 2 + (xx - OUT // 2) ** 2
    d60, d75, d90, d105 = (d2 < r for r in R2)
    return np.stack([d60, d75 & ~d60, d90 & ~d75, d105 & ~d90,
                     ~d105]).astype(np.uint8)


def _ival(mask2d):
    if not mask2d.any():
        return None
    rows = np.nonzero(mask2d.any(axis=1))[0]
    cols = np.nonzero(mask2d.any(axis=0))[0]
    return (int(rows[0]), int(rows[-1]) + 1, int(cols[0]), int(cols[-1]) + 1)


def _union(a, b):
    if a is None:
        return b
    if b is None:
        return a
    return (min(a[0], b[0]), max(a[1], b[1]),
            min(a[2], b[2]), max(a[3], b[3]))


def _plan_bands():
    regs = _regions()
    plans = []
    for it in range(NB):
        y0 = max(0, BH * it - 8)
        y1 = min(OUT, BH * it + BH - 8)
        H = y1 - y0
        ext = [[_ival(regs[r, y0:y1, wg * OW:(wg + 1) * OW])
                for wg in range(2)] for r in range(5)]
        present = [r for r in range(5) if any(e is not None for e in ext[r])]
        base = 0 if 0 in present else 4
        cps = [r for r in present if r != base]

        p = dict(it=it, y0=y0, y1=y1, H=H, ext=ext, base=base, cps=cps)

        # ---------- EE-side intervals (per wg) ----------
        if base == 4:
            s13 = [(0, H, 0, OW), (0, H, 0, OW)]
        else:
            s13 = [ext[4][wg] for wg in range(2)]
        v = [None if iv is None else
             (iv[0], iv[1], iv[2], min(iv[3] + 5, 117)) for iv in s13]
        s8 = [None if iv is None else
              (iv[0], min(iv[1] + 5, H + 5), iv[2], iv[3]) for iv in v]
        a8 = [None if iv is None else
              (iv[0], iv[1], iv[2], min(iv[3] + 4, 121)) for iv in s8]
        s7 = [None if ext[2][wg] is None else
              (ext[2][wg][0], ext[2][wg][1],
               ext[2][wg][2] + 3, ext[2][wg][3] + 3) for wg in range(2)]
        u = [None if iv is None else
             (iv[0], iv[1], iv[2], min(iv[3] + 3, 121)) for iv in s7]
        s4t = [None, None]
        for wg in range(2):
            for iv in (a8[wg], u[wg]):
                if iv is not None:
                    s4t[wg] = _union(s4t[wg],
                                     (0, H + 9, iv[2], min(iv[3], 121)))
        a4 = [None if iv is None else
              (0, H + 9, iv[2], min(iv[3] + 2, 123)) for iv in s4t]
        a2 = [None if iv is None else
              (0, H + 11, iv[2], min(iv[3] + 1, EW)) for iv in a4]

        # ---------- OO-side intervals (per wg) ----------
        s10 = [None if ext[3][wg] is None else
               (ext[3][wg][0], ext[3][wg][1],
                ext[3][wg][2] + 1, ext[3][wg][3] + 1) for wg in range(2)]
        w = [None if iv is None else
             (iv[0], iv[1], iv[2], min(iv[3] + 2, 117)) for iv in s10]
        s8o = [None if iv is None else
               (iv[0], min(iv[1] + 2, H + 2), iv[2], iv[3]) for iv in w]
        a8o = [None if iv is None else
               (iv[0], iv[1], iv[2], min(iv[3] + 4, 121)) for iv in s8o]
        s4oS = [None if ext[1][wg] is None else
                (ext[1][wg][0], ext[1][wg][1],
                 ext[1][wg][2] + 4, ext[1][wg][3] + 4) for wg in range(2)]
        s4o = [None, None]
        for wg in range(2):
            for iv in (s4oS[wg], a8o[wg]):
                if iv is not None:
                    s4o[wg] = _union(s4o[wg],
                                     (0, H + 6, iv[2], min(iv[3], 121)))
        a4o = [None if iv is None else
               (0, H + 6, iv[2], min(iv[3] + 2, 123)) for iv in s4o]
        a2o = [None if iv is None else
               (0, H + 8, iv[2], min(iv[3] + 1, EW)) for iv in a4o]
        oo_c = None
        for iv in a2o:
            if iv is not None:
                oo_c = _union(oo_c, iv)
        oo_c = None if oo_c is None else (oo_c[2], oo_c[3])

        p.update(s13=s13, v=v, s8=s8, a8=a8, s7=s7, u=u, s4t=s4t, a4=a4,
                 a2=a2, s10=s10, w=w, s8o=s8o, a8o=a8o, s4o=s4o, a4o=a4o,
                 a2o=a2o, oo_c=oo_c)
        plans.append(p)
    return plans


_PLANS = _plan_bands()


def _build_masks():
    """bm [128, NB, 4, BH, OW] u8: per band, cp-plane region masks."""
    regs = _regions()
    bm = np.zeros((128, NB, 4, BH, OW), np.uint8)
    for pl in _PLANS:
        it, y0, y1 = pl["it"], pl["y0"], pl["y1"]
        for i, r in enumerate(pl["cps"]):
            for wg in range(2):
                sl = regs[r, y0:y1, wg * OW:(wg + 1) * OW]
                bm[wg * 64:(wg + 1) * 64, it, i, 0:y1 - y0, :] = sl[None]
    return bm


def split_multi_waits(nc):
    """walrus CoreV3Gen accepts at most 1 sync-wait per instruction; Tile's
    tail drains can carry 2+.  Peel extras onto preceding NoOps."""
    n = 0
    for fn in nc.m.functions:
        for bb in fn.blocks:
            insts = list(bb.instructions)
            out = []
            for ins in insts:
                si = getattr(ins, "sync_info", None)
                if si is not None and len(si.on_wait) > 1:
                    waits = list(si.on_wait)
                    for k, w in enumerate(waits[:-1]):
                        nop = mybir.InstNoOp(
                            name=f"{ins.name}-waitsplit{k}",
                            engine=ins.engine, ins=[], outs=[])
                        nop.sync_info = mybir.SyncInfo(on_wait=[w], on_update=[])
                        out.append(nop)
                        n += 1
                    ins.sync_info = mybir.SyncInfo(
                        on_wait=[waits[-1]], on_update=list(si.on_update))
                out.append(ins)
            if n:
                bb.instructions = out
    return n


def _emit_kernel(nc: bass.Bass):
    x = nc.dram_tensor("x", [C, IN, IN], mybir.dt.float32, kind="ExternalInput")
    y = nc.dram_tensor("y", [C, OUT, OUT], mybir.dt.float32, kind="ExternalOutput")
    rmask = nc.inline_tensor(_build_masks(), name="rmask")

    dve = nc.vector
    act = nc.scalar
    mx = mybir.AluOpType.max

    with TileContext(nc) as tc:
        with tc.tile_pool(name="pp", bufs=1) as pers, \
             tc.tile_pool(name="tp", bufs=1) as tP, \
             tc.tile_pool(name="tq", bufs=1) as tQ, \
             tc.tile_pool(name="tr", bufs=2) as tR, \
             tc.tile_pool(name="to", bufs=1) as tPo, \
             tc.tile_pool(name="tqo", bufs=1) as tQo:

            # 16-row landing chunks (fp32) + fp16 cast staging
            it_bufs = [pers.tile([128, 16, WIN], mybir.dt.float32, tag=f"in{i}", name=f"itile{i}")
                       for i in range(2)]
            # deinterleaved fp16 cast staging: even cols T[2k], odd T[2k+1]
            xfe_bufs = [pers.tile([128, 16, 125], DT, tag=f"xfe{i}", name=f"xfe{i}")
                        for i in range(2)]
            xfo_bufs = [pers.tile([128, 16, 125], DT, tag=f"xfo{i}", name=f"xfo{i}")
                        for i in range(2)]
            ewt = pers.tile([128, 92, EW], DT, tag="ewt")
            owt = pers.tile([128, 92, EW], DT, tag="owt")
            ee = pers.tile([128, 45, EW], DT, tag="ee")
            oo = pers.tile([128, 43, EW], DT, tag="oo")
            s4t_t = pers.tile([128, BH + 9, 121], DT, tag="s4")
            s4o_t = pers.tile([128, BH + 6, 121], DT, tag="s4o")
            mask_t = pers.tile([128, 4, BH, OW], mybir.dt.uint8, tag="mk")
            out_bufs = [pers.tile([128, 8, OW], mybir.dt.float32, tag=f"out{i}",
                                  name=f"outt{i}") for i in range(2)]

            for itile in it_bufs:
                nc.gpsimd.memset(itile[0:64, :, 0:13], NEG)
                nc.gpsimd.memset(itile[64:128, :, 237:WIN], NEG)

            def front_thunks(it):
                """Roll + input DMAs + stage-1 pair-max + ee/oo for band
                `it`, as an ordered thunk list (DMAs interleaved ahead of
                their consumers so prefetch depth stays >= 2 chunks)."""
                pl = _PLANS[it]
                ow_iv = pl["oo_c"]
                if it + 1 < NB and _PLANS[it + 1]["oo_c"] is not None:
                    n0, n1 = _PLANS[it + 1]["oo_c"]
                    ow_iv = (min(ow_iv[0], n0), max(ow_iv[1], n1)) \
                        if ow_iv is not None else (n0, n1)

                th = []

                def roll():
                    if it == 0:
                        nc.gpsimd.memset(ewt[:, 0:28, :], NEG)
                        nc.gpsimd.memset(owt[:, 0:28, :], NEG)
                    else:
                        act.copy(ewt[:, 0:28, :], ewt[:, 64:92, :])
                        if pl["oo_c"] is not None:
                            rc0, rc1 = pl["oo_c"]
                            act.copy(owt[:, 0:28, rc0:rc1],
                                     owt[:, 64:92, rc0:rc1])
                if it == 0:
                    th.append(roll)

                def dma(ch):
                    r0 = 64 * it + 16 * ch
                    itile = it_bufs[ch % 2]
                    nc.sync.dma_start(itile[0:64, :, 13:WIN],
                                      x[:, r0:r0 + 16, 0:237])
                    nc.sync.dma_start(itile[64:128, :, 1:237],
                                      x[:, r0:r0 + 16, 212:448])

                def cast(ch):
                    # deinterleaving cast on ACT: strided reads cost the same
                    # there, and make every DVE consumer packed (2x mode)
                    act.copy(xfe_bufs[ch % 2][:, :, :],
                             it_bufs[ch % 2][:, :, 0:250:2])
                    act.copy(xfo_bufs[ch % 2][:, :, :],
                             it_bufs[ch % 2][:, :, 1:250:2])

                def comp(ch):
                    xfe = xfe_bufs[ch % 2]
                    xfo = xfo_bufs[ch % 2]
                    e0 = 28 + 16 * ch
                    # Ew[e] = max(T[2e+1], T[2e+2]) = max(odd[e], even[e+1])
                    dve.tensor_tensor(ewt[:, e0:e0 + 16, :],
                                      xfo[:, :, 0:124],
                                      xfe[:, :, 1:125], mx)
                    if ow_iv is not None:
                        c0, c1 = ow_iv
                        # Ow[e] = max(T[2e+2], T[2e+3]) = max(even[e+1], odd[e+1])
                        dve.tensor_tensor(owt[:, e0:e0 + 16, c0:c1],
                                          xfe[:, :, 1 + c0:1 + c1],
                                          xfo[:, :, 1 + c0:1 + c1], mx)

                def comp32(ch):
                    # band 0 ramp: DVE is DMA-starved anyway, so read the
                    # fp32 landing tile directly (1x) and skip the cast hop
                    itile = it_bufs[ch % 2]
                    e0 = 28 + 16 * ch
                    dve.tensor_tensor(ewt[:, e0:e0 + 16, :],
                                      itile[:, :, 1:249:2],
                                      itile[:, :, 2:250:2], mx)
                    if ow_iv is not None:
                        c0, c1 = ow_iv
                        dve.tensor_tensor(
                            owt[:, e0:e0 + 16, c0:c1],
                            itile[:, :, 2 + 2 * c0:2 + 2 * c1:2],
                            itile[:, :, 3 + 2 * c0:3 + 2 * c1:2], mx)

                rest = []
                if it == 0:
                    th.append(lambda: dma(0))
                    th.append(lambda: dma(1))
                    rest.append(lambda: comp32(0))
                    rest.append(lambda: dma(2))
                    rest.append(lambda: comp32(1))
                    rest.append(lambda: dma(3))
                    rest.append(lambda: comp32(2))
                    rest.append(lambda: comp32(3))
                elif it < 7:
                    th.append(lambda: dma(0))
                    th.append(lambda: dma(1))
                    rest.append(lambda: cast(0))
                    rest.append(lambda: dma(2))
                    rest.append(lambda: comp(0))
                    rest.append(lambda: cast(1))
                    rest.append(lambda: dma(3))
                    rest.append(lambda: comp(1))
                    rest.append(lambda: cast(2))
                    rest.append(roll)
                    rest.append(lambda: comp(2))
                    rest.append(lambda: cast(3))
                    rest.append(lambda: comp(3))
                else:
                    th.append(roll)
                    def negfill():
                        # band 7 only reads ewt rows <40 / owt rows <36
                        nc.gpsimd.memset(ewt[:, 28:40, :], NEG)
                        nc.gpsimd.memset(owt[:, 28:36, :], NEG)
                    th.append(negfill)

                def eeoo():
                    if it == 0:
                        nc.gpsimd.memset(ee[:, 0:14, :], NEG)
                        nc.gpsimd.memset(oo[:, 0:14, :], NEG)
                        rlo = 14
                    else:
                        rlo = 0
                    # a2 reads ee rows < y1+6-J0; a2o reads oo rows < y1+4-J0
                    J0 = BH * it - 14
                    eh = min(45, pl["y1"] + 6 - J0)
                    oh = min(43, pl["y1"] + 4 - J0)
                    dve.tensor_tensor(ee[:, rlo:eh, :],
                                      ewt[:, 2 * rlo:2 * eh:2, :],
                                      ewt[:, 2 * rlo + 1:2 * eh + 1:2, :], mx)
                    if pl["oo_c"] is not None:
                        c0, c1 = pl["oo_c"]
                        dve.tensor_tensor(
                            oo[:, rlo:oh, c0:c1],
                            owt[:, 2 * rlo + 1:2 * oh + 1:2, c0:c1],
                            owt[:, 2 * rlo + 2:2 * oh + 2:2, c0:c1], mx)
                return th, rest, eeoo

            def rcomb(dst, dst_base, src, src_base, jlo, jhi, d,
                      clo, chi, p0, p1):
                a = jlo - src_base
                b = jhi - src_base
                o0 = jlo - dst_base
                o1 = jhi - dst_base
                dve.tensor_tensor(dst[p0:p1, o0:o1, clo:chi],
                                  src[p0:p1, a:b, clo:chi],
                                  src[p0:p1, a + d:b + d, clo:chi], mx)

            def ccomb(dst, src, r0_, r1_, d, clo, chi, p0, p1):
                dve.tensor_tensor(dst[p0:p1, r0_:r1_, clo:chi],
                                  src[p0:p1, r0_:r1_, clo:chi],
                                  src[p0:p1, r0_:r1_, clo + d:chi + d], mx)

            def per_wg(th, ivs, fn, ovh_elems=330):
                """Append per-group emission thunks (merged into one
                128-partition op when the union is cheaper)."""
                def emit():
                    if ivs[0] is None or ivs[1] is None:
                        for wg in range(2):
                            if ivs[wg] is not None:
                                fn(wg * 64, wg * 64 + 64, ivs[wg])
                        return
                    if ivs[0] == ivs[1]:
                        fn(0, 128, ivs[0])
                        return
                    un = _union(ivs[0], ivs[1])
                    c_merge = (un[1] - un[0]) * (un[3] - un[2])
                    c_split = sum((iv[1] - iv[0]) * (iv[3] - iv[2])
                                  for iv in ivs) + ovh_elems
                    if c_merge < c_split:
                        fn(0, 128, un)
                    else:
                        for wg in range(2):
                            fn(wg * 64, wg * 64 + 64, ivs[wg])
                th.append(emit)

            def pyramid_thunks(it):
                pl = _PLANS[it]
                y0, y1, H = pl["y0"], pl["y1"], pl["H"]
                J0 = BH * it - 14
                base, cps = pl["base"], pl["cps"]
                th = []

                # ================= EE-side pyramid =================
                a2 = tP.tile([128, BH + 11, EW], DT, tag="p0")
                s2 = tQ.tile([128, BH + 11, 123], DT, tag="q0")
                a4 = tP.tile([128, BH + 11, 123], DT, tag="p0")

                per_wg(th, pl["a2"], lambda p0, p1, iv:
                       rcomb(a2, y0 - 6, ee, J0, y0 - 6, y1 + 5, 1,
                             iv[2], iv[3], p0, p1))
                per_wg(th, pl["a4"], lambda p0, p1, iv:
                       ccomb(s2, a2, 0, H + 11, 1, iv[2], iv[3], p0, p1))
                per_wg(th, pl["a4"], lambda p0, p1, iv:
                       rcomb(a4, y0 - 6, s2, y0 - 6, y0 - 6, y1 + 3, 2,
                             iv[2], iv[3], p0, p1))
                per_wg(th, pl["s4t"], lambda p0, p1, iv:
                       ccomb(s4t_t, a4, 0, H + 9, 2, iv[2], iv[3], p0, p1))

                u_t = tR.tile([128, BH, 121], DT, tag="r0")
                s7_t = tR.tile([128, BH, 118], DT, tag="r0")
                per_wg(th, pl["u"], lambda p0, p1, iv:
                       rcomb(u_t, y0 - 3, s4t_t, y0 - 6, y0 - 3 + iv[0],
                             y0 - 3 + iv[1], 3, iv[2], iv[3], p0, p1))
                per_wg(th, pl["s7"], lambda p0, p1, iv:
                       ccomb(s7_t, u_t, iv[0], iv[1], 3, iv[2],
                             min(iv[3], 118), p0, p1))

                a8_t = tP.tile([128, BH + 11, 121], DT, tag="p0")
                s8_t = tQ.tile([128, BH + 11, 117], DT, tag="q0")
                v_t = tP.tile([128, BH + 11, 117], DT, tag="p0")
                s13_t = tR.tile([128, BH, 112], DT, tag="r0")

                per_wg(th, pl["a8"], lambda p0, p1, iv:
                       rcomb(a8_t, y0 - 6, s4t_t, y0 - 6, y0 - 6 + iv[0],
                             y0 - 6 + iv[1], 4, iv[2], iv[3], p0, p1))
                per_wg(th, pl["s8"], lambda p0, p1, iv:
                       ccomb(s8_t, a8_t, iv[0], iv[1], 4, iv[2], iv[3],
                             p0, p1))
                per_wg(th, pl["v"], lambda p0, p1, iv:
                       rcomb(v_t, y0 - 6, s8_t, y0 - 6, y0 - 6 + iv[0],
                             y0 - 6 + iv[1], 5, iv[2], iv[3], p0, p1))
                per_wg(th, pl["s13"], lambda p0, p1, iv:
                       ccomb(s13_t, v_t, iv[0], iv[1], 5, iv[2],
                             min(iv[3], 112), p0, p1))

                # ================= OO-side pyramid =================
                s10_t = s7x = None
                if any(iv is not None for iv in pl["s4o"]):
                    a2o = tPo.tile([128, BH + 8, EW], DT, tag="po")
                    s2o = tQo.tile([128, BH + 8, 123], DT, tag="qo")
                    a4o = tPo.tile([128, BH + 8, 123], DT, tag="po")

                    per_wg(th, pl["a2o"], lambda p0, p1, iv:
                           rcomb(a2o, y0 - 5, oo, J0, y0 - 5, y1 + 3, 1,
                                 iv[2], iv[3], p0, p1))
                    per_wg(th, pl["a4o"], lambda p0, p1, iv:
                           ccomb(s2o, a2o, 0, H + 8, 1, iv[2], iv[3], p0, p1))
                    per_wg(th, pl["a4o"], lambda p0, p1, iv:
                           rcomb(a4o, y0 - 5, s2o, y0 - 5, y0 - 5, y1 + 1, 2,
                                 iv[2], iv[3], p0, p1))
                    per_wg(th, pl["s4o"], lambda p0, p1, iv:
                           ccomb(s4o_t, a4o, 0, H + 6, 2, iv[2], iv[3],
                                 p0, p1))

                if any(iv is not None for iv in pl["s10"]):
                    a8o = tPo.tile([128, BH + 8, 121], DT, tag="po")
                    s8o = tQo.tile([128, BH + 8, 117], DT, tag="qo")
                    w_t = tPo.tile([128, BH + 8, 117], DT, tag="po")
                    s10_t = tQo.tile([128, BH + 8, 115], DT, tag="qo")

                    per_wg(th, pl["a8o"], lambda p0, p1, iv:
                           rcomb(a8o, y0 - 5, s4o_t, y0 - 5, y0 - 5 + iv[0],
                                 y0 - 5 + iv[1], 4, iv[2], iv[3], p0, p1))
                    per_wg(th, pl["s8o"], lambda p0, p1, iv:
                           ccomb(s8o, a8o, iv[0], iv[1], 4, iv[2], iv[3],
                                 p0, p1))
                    per_wg(th, pl["w"], lambda p0, p1, iv:
                           rcomb(w_t, y0 - 5, s8o, y0 - 5, y0 - 5 + iv[0],
                                 y0 - 5 + iv[1], 2, iv[2], iv[3], p0, p1))
                    per_wg(th, pl["s10"], lambda p0, p1, iv:
                           ccomb(s10_t, w_t, iv[0], iv[1], 2, iv[2],
                                 min(iv[3], 115), p0, p1))

                # ---- masks DMA ----
                ncp = len(cps)
                if ncp:
                    th.append(lambda: nc.sync.dma_start(
                        mask_t[:, 0:ncp, 0:H, :],
                        rmask[:, it, 0:ncp, 0:H, :]))

                # ---- blend ----
                if base == 0:
                    acc_tile, acc_r0, acc_c0 = ee, y0 - J0, 6
                else:
                    acc_tile, acc_r0, acc_c0 = s13_t, 0, 0

                srcs = {0: (ee, y0 - J0, 6), 1: (s4o_t, 3, 4),
                        2: (s7_t, 0, 3), 3: (s10_t, 0, 1), 4: (s13_t, 0, 0)}
                for i, r in enumerate(cps):
                    stile, sr0, sc0 = srcs[r]

                    def em_cp(p0, p1, iv, i=i, stile=stile, sr0=sr0, sc0=sc0):
                        r0_, r1_, c0_, c1_ = iv
                        dve.copy_predicated(
                            acc_tile[p0:p1, acc_r0 + r0_:acc_r0 + r1_,
                                     acc_c0 + c0_:acc_c0 + c1_],
                            mask_t[p0:p1, i, r0_:r1_, c0_:c1_],
                            stile[p0:p1, sr0 + r0_:sr0 + r1_,
                                  sc0 + c0_:sc0 + c1_])

                    per_wg(th, pl["ext"][r], em_cp, ovh_elems=165)

                # ---- cast + store (16-row halves) ----
                def cast_store():
                    for k, h0 in enumerate(range(0, H, 8)):
                        h1 = min(h0 + 8, H)
                        out_t = out_bufs[k % 2]
                        acc = acc_tile[:, acc_r0 + h0:acc_r0 + h1,
                                       acc_c0:acc_c0 + OW]
                        act.copy(out_t[:, 0:h1 - h0, :], acc)
                        yv = y[:, y0 + h0:y0 + h1, :].rearrange(
                            "c h (w o) -> w c h o", o=OW)
                        nc.sync.dma_start(yv[0], out_t[0:64, 0:h1 - h0, :])
                        nc.sync.dma_start(yv[1], out_t[64:128, 0:h1 - h0, :])
                th.append(cast_store)
                return th

            # ---- software-pipelined emission: band it's input pipeline is
            # interleaved into band it-1's pyramid so chunk DMAs always have
            # pyramid work to hide behind ----
            lead0, rest0, eeoo0 = front_thunks(0)
            for t in lead0 + rest0:
                t()
            eeoo0()
            for it in range(1, NB):
                # ee/oo(it) overwrite tiles still read/written by band it-1's
                # pyramid+blend+cast: they must come strictly after it.
                lead, rest, eeoo = front_thunks(it)
                for t in lead:
                    t()
                pyr = pyramid_thunks(it - 1)
                k = max(1, len(pyr) // (len(rest) + 1))
                fi = 0
                for i, t in enumerate(pyr):
                    t()
                    if i % k == k - 1 and fi < len(rest):
                        rest[fi]()
                        fi += 1
                while fi < len(rest):
                    rest[fi]()
                    fi += 1
                eeoo()
            for t in pyramid_thunks(NB - 1):
                t()

    return nc


def _get_nc():
    if "nc" not in _CACHED:
        nc = bass.Bass()
        _emit_kernel(nc)
        split_multi_waits(nc)
        _CACHED["nc"] = nc
    return _CACHED["nc"]


def kernel(x: np.ndarray) -> np.ndarray:
    nc = _get_nc()
    in_maps = [{"x": np.ascontiguousarray(x[b], dtype=np.float32)}
               for b in range(B)]
    res = run_bass_kernel_spmd(nc, in_maps, core_ids=list(range(B)))
    return np.stack([r["y"] for r in res.results]).astype(np.float32)
